# revision 12
# baseline (speedup 1.0000x reference)
"""BiCut loss kernel for Trainium2, data-parallel over 8 NeuronCores.

Computes sum(output * r) / B where r[i,j] = [0.7, 0] if labels[i,j]==1
else [0, 1.3]  (alpha=0.65, r=0.5).

Strategy: shard batch dim B=8192 across 8 cores (1024 rows each). Each core
streams its 16 MiB output shard + 16 MiB int64 label shard from HBM in
[128 x 4096] tiles and fuses the masked select + reduction into three engine
ops per tile:
  DVE  tensor_tensor_reduce:  sum(0.7 * o0 * m)          (m = label in {0,1})
  DVE  tensor_tensor_reduce:  sum(-1.3 * o1 * m)
  ACT  activation(Copy,accum): sum(1.3 * o1)
so total = sum over slots. int64 labels are viewed host-side as int32 pairs
(little-endian: even words carry the 0/1 value) and only the even words feed
the multiplies (strided AP); the engines convert int32 -> f32 on read.
Per-core partial sums [128, 24] are DMA'd out and reduced on host.
"""

import os
import sys

sys.path.insert(0, "/opt/trn_rl_repo")

import numpy as np

B, L = 8192, 2048
M = 8                      # cores
BC = B // M                # 1024 rows per core
P = 128                    # SBUF partitions
NT = BC // P               # 8 row-tiles per core
ALPHA, R = 0.65, 0.5
W_POS = (1.0 - ALPHA) / R          # 0.7, weight of channel 0 when label==1
W_NEG = ALPHA / (1.0 - R)          # 1.3, weight of channel 1 when label!=1

_NC = {}
LAST = None  # last BassKernelResults, for test harness introspection


def _build(pairs, tp=126, split_rings=True, bufs=4):
    """Build the per-core program.

    pairs: labels arrive as int64 (viewed as int32 [value, 0] pairs, value
    words at stride 2) vs already-int32 (dense).
    tp: rows (partitions) per tile. 126 leaves SBUF partitions 126/127
    unused, shifting bytes off SDMA engine 15 (ports 92-95/124-127), which
    measures ~18% slower than the other engines on some cores.
    split_rings: issue label loads on the ACT HWDGE ring, output loads on
    the SP ring.
    """
    from concourse import bacc, mybir, tile

    Alu = mybir.AluOpType
    Act = mybir.ActivationFunctionType
    f32 = mybir.dt.float32
    i32 = mybir.dt.int32

    lab_cols = 2 * L if pairs else L
    ntiles = (BC + tp - 1) // tp
    nc = bacc.Bacc("TRN2", target_bir_lowering=False, debug=False)
    out_d = nc.dram_tensor("out_f", [BC, 2 * L], f32, kind="ExternalInput")
    lab_d = nc.dram_tensor("lab_i", [BC, lab_cols], i32, kind="ExternalInput")
    acc_d = nc.dram_tensor("acc_out", [P, 3 * ntiles], f32, kind="ExternalOutput")
    lab_ring = nc.scalar if split_rings else nc.sync

    with tile.TileContext(nc) as tc:
        with tc.tile_pool(name="io", bufs=bufs) as io, \
             tc.tile_pool(name="sc", bufs=2) as sc, \
             tc.tile_pool(name="accp", bufs=1) as accp:
            acc_v = accp.tile([P, 2 * ntiles], f32)   # DVE accum slots
            acc_s = accp.tile([P, ntiles], f32)       # ACT accum slots
            nc.vector.memset(acc_v, 0.0)
            nc.vector.memset(acc_s, 0.0)
            for t in range(ntiles):
                r0 = t * tp
                np_ = min(tp, BC - r0)
                g = io.tile([P, 2 * L], f32, tag="g")
                lb = io.tile([P, lab_cols], i32, tag="lb")
                nc.sync.dma_start(out=g[0:np_, :],
                                  in_=out_d.ap()[r0:r0 + np_, :])
                lab_ring.dma_start(out=lb[0:np_, :],
                                   in_=lab_d.ap()[r0:r0 + np_, :])
                gv = g[0:np_, :].rearrange("p (j c) -> p j c", c=2)
                o0 = gv[:, :, 0]
                o1 = gv[:, :, 1]
                if pairs:
                    m = lb[0:np_, :].rearrange("p (j c) -> p j c", c=2)[:, :, 0]
                else:
                    m = lb[0:np_, :]
                s0 = sc.tile([P, L], f32, tag="s0")
                s1 = sc.tile([P, L], f32, tag="s1")
                s2 = sc.tile([P, L], f32, tag="s2")
                nc.vector.scalar_tensor_tensor(
                    out=s0[0:np_, :], in0=o0, scalar=W_POS, in1=m,
                    op0=Alu.mult, op1=Alu.mult,
                    accum_out=acc_v[0:np_, 2 * t:2 * t + 1],
                )
                nc.vector.scalar_tensor_tensor(
                    out=s1[0:np_, :], in0=o1, scalar=-W_NEG, in1=m,
                    op0=Alu.mult, op1=Alu.mult,
                    accum_out=acc_v[0:np_, 2 * t + 1:2 * t + 2],
                )
                nc.scalar.activation(
                    out=s2[0:np_, :], in_=o1, func=Act.Copy, scale=W_NEG,
                    accum_out=acc_s[0:np_, t:t + 1],
                )
            nc.sync.dma_start(out=acc_d.ap()[:, 0:2 * ntiles], in_=acc_v)
            lab_ring.dma_start(out=acc_d.ap()[:, 2 * ntiles:3 * ntiles],
                               in_=acc_s)
    nc.finalize()
    return nc


def _config():
    return (
        int(os.environ.get("BICUT_TP", "126")),
        bool(int(os.environ.get("BICUT_SPLIT", "1"))),
        int(os.environ.get("BICUT_BUFS", "4")),
    )


def _get_nc(pairs):
    key = (pairs, *_config())
    if key not in _NC:
        tp, split, bufs = _config()
        _NC[key] = _build(pairs, tp=tp, split_rings=split, bufs=bufs)
    return _NC[key]


def _ensure_ntff_hook():
    """The image's antenv package lacks axon_hooks; synthesize it and wire
    the ctypes NTFF-profiling hook so run_bass_kernel_spmd(trace=True)
    can capture HW exec times under axon."""
    import types

    try:
        import antenv.axon_hooks  # noqa: F401
        return
    except ImportError:
        pass
    import antenv

    mod = types.ModuleType("antenv.axon_hooks")
    mod._hook = None
    mod.set_axon_ntff_profile_hook = lambda h: setattr(mod, "_hook", h)
    mod.get_axon_ntff_profile_hook = lambda: mod._hook
    sys.modules["antenv.axon_hooks"] = mod
    antenv.axon_hooks = mod
    try:
        from trn_agent_boot.trn_boot import _ntff_profile_via_ctypes

        mod._hook = _ntff_profile_via_ctypes("/opt/axon/libaxon_pjrt.so")
    except Exception:
        pass


def _run(in_maps, pairs, trace=False):
    global LAST
    from concourse import bass_utils

    if trace:
        _ensure_ntff_hook()
        # artifact upload needs external storage; keep artifacts local
        bass_utils.upload_artifacts = lambda tmpdir: tmpdir

    LAST = bass_utils.run_bass_kernel_spmd(
        _get_nc(pairs), in_maps, core_ids=list(range(M)), trace=trace
    )
    return LAST


def kernel(output, labels):
    output = np.asarray(output)
    labels = np.asarray(labels)
    assert output.shape == (B, L, 2), output.shape
    assert labels.shape == (B, L), labels.shape
    out_f = np.ascontiguousarray(output).astype(np.float32, copy=False)
    out_f = out_f.reshape(B, 2 * L)
    if labels.dtype == np.int64:
        # int64 -> int32 pairs; little-endian, so even words hold the value
        pairs = True
        lab_i = np.ascontiguousarray(labels).view(np.int32).reshape(B, 2 * L)
    else:
        pairs = False
        lab_i = np.ascontiguousarray(labels).astype(np.int32, copy=False)
        lab_i = lab_i.reshape(B, L)

    in_maps = [
        {
            "out_f": out_f[k * BC:(k + 1) * BC],
            "lab_i": lab_i[k * BC:(k + 1) * BC],
        }
        for k in range(M)
    ]
    trace = bool(int(os.environ.get("BICUT_TRACE", "0")))
    res = _run(in_maps, pairs, trace=trace)
    total = 0.0
    for r in res.results:
        total += r["acc_out"].sum(dtype=np.float64)
    return np.array(total / B, dtype=np.float32)


# revision 14
# speedup vs baseline: 1.2042x; 1.2042x over previous
"""BiCut loss kernel for Trainium2, data-parallel over 8 NeuronCores.

Computes sum(output * r) / B where r[i,j] = [0.7, 0] if labels[i,j]==1
else [0, 1.3]  (alpha=0.65, r=0.5).

Strategy: shard batch dim B=8192 across 8 cores (1024 rows each). Each core
streams its 16 MiB output shard + 16 MiB int64 label shard from HBM in
[128 x 4096] tiles and fuses the masked select + reduction into three engine
ops per tile:
  DVE  tensor_tensor_reduce:  sum(0.7 * o0 * m)          (m = label in {0,1})
  DVE  tensor_tensor_reduce:  sum(-1.3 * o1 * m)
  ACT  activation(Copy,accum): sum(1.3 * o1)
so total = sum over slots. int64 labels are viewed host-side as int32 pairs
(little-endian: even words carry the 0/1 value) and only the even words feed
the multiplies (strided AP); the engines convert int32 -> f32 on read.
Per-core partial sums [128, 24] are DMA'd out and reduced on host.
"""

import os
import sys

sys.path.insert(0, "/opt/trn_rl_repo")

import numpy as np

B, L = 8192, 2048
M = 8                      # cores
BC = B // M                # 1024 rows per core
P = 128                    # SBUF partitions
NT = BC // P               # 8 row-tiles per core
ALPHA, R = 0.65, 0.5
W_POS = (1.0 - ALPHA) / R          # 0.7, weight of channel 0 when label==1
W_NEG = ALPHA / (1.0 - R)          # 1.3, weight of channel 1 when label!=1

_NC = {}
LAST = None  # last BassKernelResults, for test harness introspection


def _build(pairs, tp=128, split_rings=False, bufs=4, cs=2):
    """Build the per-core program.

    pairs: labels arrive as int64 (viewed as int32 [value, 0] pairs, value
    words at stride 2) vs already-int32 (dense).
    tp: rows (partitions) per tile. Must stay 128: partial-partition DMAs
    collapse to fewer SDMA engines and lose ~40% bandwidth (measured).
    split_rings: issue label loads on the ACT HWDGE ring (measured worse:
    DMA issue serializes behind ACT compute).
    cs: column chunks per row-tile. 2 halves the last-chunk compute tail
    and lets compute start after half a tile has landed.
    """
    from concourse import bacc, mybir, tile

    Alu = mybir.AluOpType
    Act = mybir.ActivationFunctionType
    f32 = mybir.dt.float32
    i32 = mybir.dt.int32

    lab_cols = 2 * L if pairs else L
    ntiles = BC // tp
    nch = ntiles * cs              # total chunks
    gw = 2 * L // cs               # output cols per chunk
    lw = lab_cols // cs            # label cols per chunk
    jw = L // cs                   # pairs per chunk
    nc = bacc.Bacc("TRN2", target_bir_lowering=False, debug=False)
    out_d = nc.dram_tensor("out_f", [BC, 2 * L], f32, kind="ExternalInput")
    lab_d = nc.dram_tensor("lab_i", [BC, lab_cols], i32, kind="ExternalInput")
    acc_d = nc.dram_tensor("acc_out", [P, 3 * nch], f32, kind="ExternalOutput")
    lab_ring = nc.scalar if split_rings else nc.sync
    ap_out = out_d.ap()
    ap_lab = lab_d.ap()
    ap_acc = acc_d.ap()

    with tile.TileContext(nc) as tc:
        with tc.tile_pool(name="io", bufs=bufs) as io, \
             tc.tile_pool(name="sc", bufs=2) as sc, \
             tc.tile_pool(name="accp", bufs=1) as accp:
            # disjoint early/late accum tiles so draining the early slots
            # can't create WAR hazards with the final chunk's writes
            ne = nch - 1
            accv_e = accp.tile([P, 2 * ne], f32)
            accv_l = accp.tile([P, 2], f32)
            accs_e = accp.tile([P, ne], f32)
            accs_l = accp.tile([P, 1], f32)
            for i in range(nch):
                t, c = divmod(i, cs)
                r0 = t * tp
                last = i == nch - 1
                g = io.tile([P, gw], f32, tag="g")
                lb = io.tile([P, lw], i32, tag="lb")
                nc.sync.dma_start(
                    out=g, in_=ap_out[r0:r0 + tp, c * gw:(c + 1) * gw])
                lab_ring.dma_start(
                    out=lb, in_=ap_lab[r0:r0 + tp, c * lw:(c + 1) * lw])
                gv = g.rearrange("p (j c) -> p j c", c=2)
                o0 = gv[:, :, 0]
                o1 = gv[:, :, 1]
                if pairs:
                    m = lb.rearrange("p (j c) -> p j c", c=2)[:, :, 0]
                else:
                    m = lb[:, :]
                s0 = sc.tile([P, jw], f32, tag="s0")
                s1 = sc.tile([P, jw], f32, tag="s1")
                s2 = sc.tile([P, jw], f32, tag="s2")
                av = accv_l if last else accv_e
                asl = accs_l if last else accs_e
                k = 0 if last else i
                nc.vector.scalar_tensor_tensor(
                    out=s0, in0=o0, scalar=W_POS, in1=m,
                    op0=Alu.mult, op1=Alu.mult,
                    accum_out=av[:, 2 * k:2 * k + 1],
                )
                nc.vector.scalar_tensor_tensor(
                    out=s1, in0=o1, scalar=-W_NEG, in1=m,
                    op0=Alu.mult, op1=Alu.mult,
                    accum_out=av[:, 2 * k + 1:2 * k + 2],
                )
                nc.scalar.activation(
                    out=s2, in_=o1, func=Act.Copy, scale=W_NEG,
                    accum_out=asl[:, k:k + 1],
                )
            nc.sync.dma_start(out=ap_acc[:, 0:2 * ne], in_=accv_e)
            nc.sync.dma_start(out=ap_acc[:, 2 * ne:2 * ne + ne], in_=accs_e)
            nc.sync.dma_start(out=ap_acc[:, 3 * ne:3 * ne + 2], in_=accv_l)
            nc.sync.dma_start(out=ap_acc[:, 3 * ne + 2:3 * ne + 3], in_=accs_l)
    nc.finalize()
    return nc


def _config():
    return (
        int(os.environ.get("BICUT_TP", "128")),
        bool(int(os.environ.get("BICUT_SPLIT", "0"))),
        int(os.environ.get("BICUT_BUFS", "4")),
        int(os.environ.get("BICUT_CS", "2")),
    )


def _get_nc(pairs):
    key = (pairs, *_config())
    if key not in _NC:
        tp, split, bufs, cs = _config()
        _NC[key] = _build(pairs, tp=tp, split_rings=split, bufs=bufs, cs=cs)
    return _NC[key]


def _ensure_ntff_hook():
    """The image's antenv package lacks axon_hooks; synthesize it and wire
    the ctypes NTFF-profiling hook so run_bass_kernel_spmd(trace=True)
    can capture HW exec times under axon."""
    import types

    try:
        import antenv.axon_hooks  # noqa: F401
        return
    except ImportError:
        pass
    import antenv

    mod = types.ModuleType("antenv.axon_hooks")
    mod._hook = None
    mod.set_axon_ntff_profile_hook = lambda h: setattr(mod, "_hook", h)
    mod.get_axon_ntff_profile_hook = lambda: mod._hook
    sys.modules["antenv.axon_hooks"] = mod
    antenv.axon_hooks = mod
    try:
        from trn_agent_boot.trn_boot import _ntff_profile_via_ctypes

        mod._hook = _ntff_profile_via_ctypes("/opt/axon/libaxon_pjrt.so")
    except Exception:
        pass


def _run(in_maps, pairs, trace=False):
    global LAST
    from concourse import bass_utils

    if trace:
        _ensure_ntff_hook()
        # artifact upload needs external storage; keep artifacts local
        bass_utils.upload_artifacts = lambda tmpdir: tmpdir

    LAST = bass_utils.run_bass_kernel_spmd(
        _get_nc(pairs), in_maps, core_ids=list(range(M)), trace=trace
    )
    return LAST


def kernel(output, labels):
    output = np.asarray(output)
    labels = np.asarray(labels)
    assert output.shape == (B, L, 2), output.shape
    assert labels.shape == (B, L), labels.shape
    out_f = np.ascontiguousarray(output).astype(np.float32, copy=False)
    out_f = out_f.reshape(B, 2 * L)
    if labels.dtype == np.int64:
        # int64 -> int32 pairs; little-endian, so even words hold the value
        pairs = True
        lab_i = np.ascontiguousarray(labels).view(np.int32).reshape(B, 2 * L)
    else:
        pairs = False
        lab_i = np.ascontiguousarray(labels).astype(np.int32, copy=False)
        lab_i = lab_i.reshape(B, L)

    in_maps = [
        {
            "out_f": out_f[k * BC:(k + 1) * BC],
            "lab_i": lab_i[k * BC:(k + 1) * BC],
        }
        for k in range(M)
    ]
    trace = bool(int(os.environ.get("BICUT_TRACE", "0")))
    res = _run(in_maps, pairs, trace=trace)
    total = 0.0
    for r in res.results:
        total += r["acc_out"].sum(dtype=np.float64)
    return np.array(total / B, dtype=np.float32)


# revision 21
# speedup vs baseline: 1.2361x; 1.0265x over previous
"""BiCut loss kernel for Trainium2, data-parallel over 8 NeuronCores.

Computes sum(output * r) / B where r[i,j] = [0.7, 0] if labels[i,j]==1
else [0, 1.3]  (alpha=0.65, r=0.5).

Strategy: shard batch dim B=8192 across 8 cores (1024 rows each). Each core
streams its 16 MiB output shard + its label shard from HBM in full
128-partition chunks and fuses the masked select + reduction into three
engine ops per chunk (m = label value in {0,1}):
  DVE  scalar_tensor_tensor: sum((o0 * 0.7) * m)   -> accum slot
  DVE  scalar_tensor_tensor: sum((o1 * -1.3) * m)  -> accum slot
  ACT  activation(Copy, scale=1.3, accum_out): sum(1.3 * o1)
since per-element loss = 0.7*o0*m + 1.3*o1*(1-m). int64 labels are viewed
host-side as int32 pairs (little-endian: even words carry the 0/1 value) and
only the even words feed the multiplies (strided AP); the engines convert
int32 -> f32 on read. Per-partition accum slots are DMA'd out (early slots
drained while the tail still computes) and reduced on host in float64.

Measured (trace=1, all-core NTFF): ~80 us mean / ~94 us max across the 8
cores with int32 labels — at the chip HBM roofline (8 cores x 24 MiB at
~2.9 TB/s aggregate, plus ~7 us fixed NEFF preamble per core).
"""

import os
import sys

sys.path.insert(0, "/opt/trn_rl_repo")

import numpy as np

B, L = 8192, 2048
M = 8                      # cores
BC = B // M                # 1024 rows per core
P = 128                    # SBUF partitions
NT = BC // P               # 8 row-tiles per core
ALPHA, R = 0.65, 0.5
W_POS = (1.0 - ALPHA) / R          # 0.7, weight of channel 0 when label==1
W_NEG = ALPHA / (1.0 - R)          # 1.3, weight of channel 1 when label!=1

_NC = {}
LAST = None  # last BassKernelResults, for test harness introspection


def _build(pairs, tp=128, split_rings=False, bufs=4, cs=2, fold=1):
    """Build the per-core program.

    pairs: labels arrive as int64 (viewed as int32 [value, 0] pairs, value
    words at stride 2) vs already-int32 (dense).
    tp: rows (partitions) per tile. Must stay 128: partial-partition DMAs
    collapse to fewer SDMA engines and lose ~40% bandwidth (measured).
    split_rings: issue label loads on the ACT HWDGE ring (measured worse:
    DMA issue serializes behind ACT compute).
    cs: column chunks per row-tile. 2 halves the last-chunk compute tail
    and lets compute start after half a tile has landed.
    """
    from concourse import bacc, mybir, tile

    Alu = mybir.AluOpType
    Act = mybir.ActivationFunctionType
    f32 = mybir.dt.float32
    i32 = mybir.dt.int32

    # fold: DRAM rows per SBUF partition; >1 doubles descriptor size and
    # halves dma_start count for the same bytes (pure host-side reshape)
    lab_cols = (2 * L if pairs else L) * fold
    rows = BC // fold
    rcols = 2 * L * fold
    assert rows % tp == 0 and rcols % (2 * cs) == 0 and lab_cols % cs == 0
    ntiles = rows // tp
    nch = ntiles * cs              # total chunks
    gw = rcols // cs               # output cols per chunk
    lw = lab_cols // cs            # label cols per chunk
    jw = rcols // 2 // cs          # pairs per chunk
    nc = bacc.Bacc("TRN2", target_bir_lowering=False, debug=False)
    out_d = nc.dram_tensor("out_f", [rows, rcols], f32, kind="ExternalInput")
    lab_d = nc.dram_tensor("lab_i", [rows, lab_cols], i32, kind="ExternalInput")
    acc_d = nc.dram_tensor("acc_out", [P, 3 * nch], f32, kind="ExternalOutput")
    lab_ring = nc.scalar if split_rings else nc.sync
    ap_out = out_d.ap()
    ap_lab = lab_d.ap()
    ap_acc = acc_d.ap()

    with tile.TileContext(nc) as tc:
        with tc.tile_pool(name="io", bufs=bufs) as io, \
             tc.tile_pool(name="sc", bufs=2) as sc, \
             tc.tile_pool(name="accp", bufs=1) as accp:
            # disjoint early/late accum tiles so draining the early slots
            # can't create WAR hazards with the final chunk's writes
            ne = nch - 1
            accv_e = accp.tile([P, 2 * ne], f32)
            accv_l = accp.tile([P, 2], f32)
            accs_e = accp.tile([P, ne], f32)
            accs_l = accp.tile([P, 1], f32)
            for i in range(nch):
                t, c = divmod(i, cs)
                r0 = t * tp
                last = i == nch - 1
                g = io.tile([P, gw], f32, tag="g")
                lb = io.tile([P, lw], i32, tag="lb")
                nc.sync.dma_start(
                    out=g, in_=ap_out[r0:r0 + tp, c * gw:(c + 1) * gw])
                lab_ring.dma_start(
                    out=lb, in_=ap_lab[r0:r0 + tp, c * lw:(c + 1) * lw])
                gv = g.rearrange("p (j c) -> p j c", c=2)
                o0 = gv[:, :, 0]
                o1 = gv[:, :, 1]
                if pairs:
                    m = lb.rearrange("p (j c) -> p j c", c=2)[:, :, 0]
                else:
                    m = lb[:, :]
                s0 = sc.tile([P, jw], f32, tag="s0")
                s1 = sc.tile([P, jw], f32, tag="s1")
                s2 = sc.tile([P, jw], f32, tag="s2")
                av = accv_l if last else accv_e
                asl = accs_l if last else accs_e
                k = 0 if last else i
                nc.vector.scalar_tensor_tensor(
                    out=s0, in0=o0, scalar=W_POS, in1=m,
                    op0=Alu.mult, op1=Alu.mult,
                    accum_out=av[:, 2 * k:2 * k + 1],
                )
                nc.vector.scalar_tensor_tensor(
                    out=s1, in0=o1, scalar=-W_NEG, in1=m,
                    op0=Alu.mult, op1=Alu.mult,
                    accum_out=av[:, 2 * k + 1:2 * k + 2],
                )
                nc.scalar.activation(
                    out=s2, in_=o1, func=Act.Copy, scale=W_NEG,
                    accum_out=asl[:, k:k + 1],
                )
            nc.sync.dma_start(out=ap_acc[:, 0:2 * ne], in_=accv_e)
            nc.sync.dma_start(out=ap_acc[:, 2 * ne:2 * ne + ne], in_=accs_e)
            nc.sync.dma_start(out=ap_acc[:, 3 * ne:3 * ne + 2], in_=accv_l)
            nc.sync.dma_start(out=ap_acc[:, 3 * ne + 2:3 * ne + 3], in_=accs_l)
    nc.finalize()
    return nc


def _config():
    return (
        int(os.environ.get("BICUT_TP", "128")),
        bool(int(os.environ.get("BICUT_SPLIT", "0"))),
        int(os.environ.get("BICUT_BUFS", "4")),
        int(os.environ.get("BICUT_CS", "2")),
        int(os.environ.get("BICUT_FOLD", "2")),
    )


def _get_nc(pairs):
    key = (pairs, *_config())
    if key not in _NC:
        tp, split, bufs, cs, fold = _config()
        _NC[key] = _build(pairs, tp=tp, split_rings=split, bufs=bufs, cs=cs,
                          fold=fold)
    return _NC[key]


def _ensure_ntff_hook():
    """The image's antenv package lacks axon_hooks; synthesize it and wire
    the ctypes NTFF-profiling hook so run_bass_kernel_spmd(trace=True)
    can capture HW exec times under axon."""
    import types

    try:
        import antenv.axon_hooks  # noqa: F401
        return
    except ImportError:
        pass
    import antenv

    mod = types.ModuleType("antenv.axon_hooks")
    mod._hook = None
    mod.set_axon_ntff_profile_hook = lambda h: setattr(mod, "_hook", h)
    mod.get_axon_ntff_profile_hook = lambda: mod._hook
    sys.modules["antenv.axon_hooks"] = mod
    antenv.axon_hooks = mod
    try:
        from trn_agent_boot.trn_boot import _ntff_profile_via_ctypes

        mod._hook = _ntff_profile_via_ctypes("/opt/axon/libaxon_pjrt.so")
    except Exception:
        pass


def _run(in_maps, pairs, trace=False):
    global LAST
    from concourse import bass_utils

    if trace:
        _ensure_ntff_hook()
        # artifact upload needs external storage; keep artifacts local
        bass_utils.upload_artifacts = lambda tmpdir: tmpdir

    LAST = bass_utils.run_bass_kernel_spmd(
        _get_nc(pairs), in_maps, core_ids=list(range(M)), trace=trace
    )
    return LAST


def kernel(output, labels):
    output = np.asarray(output)
    labels = np.asarray(labels)
    assert output.shape == (B, L, 2), output.shape
    assert labels.shape == (B, L), labels.shape
    out_f = np.ascontiguousarray(output).astype(np.float32, copy=False)
    out_f = out_f.reshape(B, 2 * L)
    if labels.dtype == np.int64:
        # int64 -> int32 pairs; little-endian, so even words hold the value
        pairs = True
        lab_i = np.ascontiguousarray(labels).view(np.int32).reshape(B, 2 * L)
    else:
        pairs = False
        lab_i = np.ascontiguousarray(labels).astype(np.int32, copy=False)
        lab_i = lab_i.reshape(B, L)

    fold = _config()[4]
    lc = lab_i.shape[1]
    in_maps = [
        {
            "out_f": out_f[k * BC:(k + 1) * BC].reshape(BC // fold,
                                                        2 * L * fold),
            "lab_i": lab_i[k * BC:(k + 1) * BC].reshape(BC // fold,
                                                        lc * fold),
        }
        for k in range(M)
    ]
    trace = bool(int(os.environ.get("BICUT_TRACE", "0")))
    res = _run(in_maps, pairs, trace=trace)
    total = 0.0
    for r in res.results:
        total += r["acc_out"].sum(dtype=np.float64)
    return np.array(total / B, dtype=np.float32)
